# revision 1
# baseline (speedup 1.0000x reference)
"""GAT 2-layer + BN + classifier on 8 TRN2 NeuronCores (Bass/Tile).

Strategy: dst-block sharding with per-core table rotation so the SPMD
instruction stream is identical across cores. 5 launches:
  L1   node: h1_aug = x @ [W1 | W1@as1 | W1@ad1], table rows [h|as|ad|1|0]
  E(1) edge: gather h1_aug[src] per edge subtile, segment softmax via
       selection-matrix matmul in PSUM, partial BN stats
  L3   node: BN1 apply + relu + @W2_aug -> h2_aug table shard
  E(2) edge: same NEFF as E(1) on h2_aug
  L5   node: BN2 apply + relu + @Wc + bc -> logits shard
Host work is index-only: edge sort/shard, table assembly/rotation.
"""
import sys
sys.path.insert(0, '/opt/trn_rl_repo')
sys.path.insert(0, '/root/.axon_site')
import numpy as np

import concourse.bass as bass
import concourse.bacc as bacc
import concourse.tile as tile
from concourse import mybir
from concourse.masks import make_identity

F32 = mybir.dt.float32
I32 = mybir.dt.int32

N = 100000
NCORE = 8
BLK = 128
NPAD = 100352            # 784 blocks of 128
PC = NPAD // NCORE       # 12544 nodes/core = 98 blocks
NBLK = PC // BLK         # 98
TCOLS = 132              # table row: [h(128) | as | ad | one | pad]
HID = 128
NCLS = 40
NEG = 0.2
EPS = 1e-5

_EXEC_NS = []            # per-launch exec times when profiling enabled
PROFILE = False


RUN_HOOK = None          # test harness may set this to a profiling runner


def _run(nc, in_maps, label):
    if RUN_HOOK is not None:
        return RUN_HOOK(nc, in_maps, label)
    from concourse import bass2jax
    return bass2jax.run_bass_via_pjrt(nc, in_maps, n_cores=NCORE)


# ---------------------------------------------------------------- L1 node
def build_l1():
    nc = bacc.Bacc("TRN2", target_bir_lowering=False, debug=False, num_devices=NCORE)
    xT = nc.dram_tensor("xT", [128, NPAD], F32, kind="ExternalInput")
    W1 = nc.dram_tensor("W1", [128, HID], F32, kind="ExternalInput")
    avec = nc.dram_tensor("avec", [128, 2], F32, kind="ExternalInput")  # [as|ad] cols
    out = nc.dram_tensor("out", [PC, TCOLS], F32, kind="ExternalOutput")

    with tile.TileContext(nc) as tc:
        with (
            tc.tile_pool(name="c", bufs=1) as cp,
            tc.tile_pool(name="x", bufs=3) as xp,
            tc.tile_pool(name="o", bufs=3) as op,
            tc.tile_pool(name="ps", bufs=2, space="PSUM") as pp,
            tc.tile_pool(name="ps1", bufs=1, space="PSUM") as pp1,
        ):
            ident = cp.tile([128, 128], F32)
            make_identity(nc, ident[:])
            w_sb = cp.tile([128, HID], F32)
            nc.sync.dma_start(w_sb[:], W1[:])
            av_sb = cp.tile([128, 2], F32)
            nc.sync.dma_start(av_sb[:], avec[:])
            # W1T for v = W1 @ a
            wT_ps = pp1.tile([128, 128], F32, tag="tmp")
            nc.tensor.transpose(out=wT_ps[:], in_=w_sb[:], identity=ident[:])
            wT_sb = cp.tile([128, 128], F32)
            nc.vector.tensor_copy(out=wT_sb[:], in_=wT_ps[:])
            v_ps = pp1.tile([128, 2], F32, tag="tmp")
            nc.tensor.matmul(out=v_ps[:], lhsT=wT_sb[:], rhs=av_sb[:], start=True, stop=True)
            waug = cp.tile([128, HID + 2], F32)
            nc.vector.tensor_copy(out=waug[:, 0:HID], in_=w_sb[:])
            nc.vector.tensor_copy(out=waug[:, HID:HID + 2], in_=v_ps[:])
            GB = 4
            for g0 in range(0, NBLK, GB):
                nb = min(GB, NBLK - g0)
                xs = xp.tile([128, GB * 128], F32, tag="x", name=f"xs{g0}")
                nc.sync.dma_start(xs[:, 0:nb * 128],
                                  xT[:, g0 * 128:(g0 + nb) * 128])
                for i in range(nb):
                    t = g0 + i
                    h_ps = pp.tile([128, HID + 2], F32, tag="h", name=f"h{t}")
                    nc.tensor.matmul(out=h_ps[:], lhsT=xs[:, i * 128:(i + 1) * 128],
                                     rhs=waug[:], start=True, stop=True)
                    ot = op.tile([128, TCOLS], F32, tag="o", name=f"o{t}")
                    nc.vector.tensor_copy(out=ot[:, 0:HID + 2], in_=h_ps[:])
                    nc.vector.memset(ot[:, HID + 2:HID + 3], 1.0)
                    nc.vector.memset(ot[:, HID + 3:TCOLS], 0.0)
                    nc.scalar.dma_start(out[t * 128:(t + 1) * 128, :], ot[:])
    nc.compile()
    return nc


# ---------------------------------------------------------------- edge kernel
def build_edge(t_counts):
    """t_counts: list of NBLK subtile counts (shared across cores)."""
    nsub = int(sum(t_counts))
    nc = bacc.Bacc("TRN2", target_bir_lowering=False, debug=False, num_devices=NCORE)
    table = nc.dram_tensor("table", [NPAD, TCOLS], F32, kind="ExternalInput")
    src_idx = nc.dram_tensor("src_idx", [128, nsub], I32, kind="ExternalInput")
    dst_loc = nc.dram_tensor("dst_loc", [128, nsub], F32, kind="ExternalInput")
    agg = nc.dram_tensor("agg", [PC, HID], F32, kind="ExternalOutput")
    stats = nc.dram_tensor("stats", [1, 256], F32, kind="ExternalOutput")

    TMAX = max(t_counts)
    with tile.TileContext(nc) as tc:
        with (
            tc.tile_pool(name="c", bufs=1) as cp,
            tc.tile_pool(name="g", bufs=24) as gp,
            tc.tile_pool(name="s0", bufs=24) as s0p,
            tc.tile_pool(name="sw", bufs=3) as swp,
            tc.tile_pool(name="w", bufs=2) as wp,
            tc.tile_pool(name="ob", bufs=3) as obp,
            tc.tile_pool(name="own", bufs=2) as ownp,
            tc.tile_pool(name="pblk", bufs=2, space="PSUM") as pblk,
            tc.tile_pool(name="pal", bufs=2, space="PSUM") as pal,
            tc.tile_pool(name="ptr", bufs=2, space="PSUM") as ptr,
            tc.tile_pool(name="pst", bufs=1, space="PSUM") as pst,
        ):
            ident = cp.tile([128, 128], F32)
            make_identity(nc, ident[:])
            iota_i = cp.tile([128, 128], I32)
            nc.gpsimd.iota(iota_i[:], pattern=[[1, 128]], base=0, channel_multiplier=0)
            iota_f = cp.tile([128, 128], F32)
            nc.vector.tensor_copy(out=iota_f[:], in_=iota_i[:])
            ones_col = cp.tile([128, 1], F32)
            nc.vector.memset(ones_col[:], 1.0)
            idx_sb = cp.tile([128, nsub], I32)
            nc.sync.dma_start(idx_sb[:], src_idx[:])
            dl_sb = cp.tile([128, nsub], F32)
            nc.sync.dma_start(dl_sb[:], dst_loc[:])

            ps_sum = pst.tile([1, 128], F32, tag="sum")
            ps_sq = pst.tile([1, 128], F32, tag="sq")

            # zero-init gather buffers (stale-data guard)
            for i in range(24):
                gi = gp.tile([128, TCOLS], F32, tag="g", name=f"ginit{i}")
                nc.vector.memset(gi[:], 0.0)

            q0 = 0
            for t in range(NBLK):
                T = t_counts[t]
                # own rows: alpha_d of the block's nodes (cols 128..132)
                ownt = ownp.tile([128, 4], F32, tag="own")
                nc.sync.dma_start(ownt[:], table[t * 128:(t + 1) * 128, HID:HID + 4])
                ps_a = pal.tile([128, TMAX], F32, tag="al")
                s0_list = []
                g_list = []
                for s in range(T):
                    q = q0 + s
                    g = gp.tile([128, TCOLS], F32, tag="g")
                    nc.gpsimd.indirect_dma_start(
                        out=g[:], out_offset=None, in_=table[:],
                        in_offset=bass.IndirectOffsetOnAxis(ap=idx_sb[:, q:q + 1], axis=0))
                    g_list.append(g)
                    s0 = s0p.tile([128, 128], F32, tag="s0")
                    nc.vector.tensor_scalar(
                        out=s0[:], in0=iota_f[:], scalar1=dl_sb[:, q:q + 1],
                        scalar2=None, op0=mybir.AluOpType.is_equal)
                    s0_list.append(s0)
                    s0t_ps = ptr.tile([128, 128], F32, tag="tr")
                    nc.tensor.transpose(out=s0t_ps[:], in_=s0[:], identity=ident[:])
                    s0t = swp.tile([128, 128], F32, tag="s0t")
                    nc.vector.tensor_copy(out=s0t[:], in_=s0t_ps[:])
                    # alpha_d per edge -> ps_a[:, s]
                    nc.tensor.matmul(out=ps_a[:, s:s + 1], lhsT=s0t[:],
                                     rhs=ownt[:, 1:2], start=True, stop=True)
                # w = exp(lrelu(alpha_s + alpha_d)) for the whole block
                w_blk = wp.tile([128, TMAX], F32, tag="w")
                ps_b = pblk.tile([128, HID + 3], F32, tag="blk")
                for s in range(T):
                    g = g_list[s]
                    # alpha = alpha_s (g col 128) + alpha_d (ps_a col s)
                    nc.vector.tensor_tensor(
                        out=w_blk[:, s:s + 1], in0=g[:, HID:HID + 1],
                        in1=ps_a[:, s:s + 1], op=mybir.AluOpType.add)
                wb2 = wp.tile([128, TMAX], F32, tag="w2")
                nc.vector.tensor_scalar(out=wb2[:, 0:T], in0=w_blk[:, 0:T],
                                        scalar1=NEG, scalar2=None,
                                        op0=mybir.AluOpType.mult)
                nc.vector.tensor_tensor(out=w_blk[:, 0:T], in0=w_blk[:, 0:T],
                                        in1=wb2[:, 0:T], op=mybir.AluOpType.max)
                nc.scalar.activation(out=w_blk[:, 0:T], in_=w_blk[:, 0:T],
                                     func=mybir.ActivationFunctionType.Exp)
                for s in range(T):
                    sw = swp.tile([128, 128], F32, tag="sw")
                    nc.vector.tensor_scalar(
                        out=sw[:], in0=s0_list[s][:], scalar1=w_blk[:, s:s + 1],
                        scalar2=None, op0=mybir.AluOpType.mult)
                    nc.tensor.matmul(out=ps_b[:], lhsT=sw[:], rhs=g_list[s][:, 0:HID + 3],
                                     start=(s == 0), stop=(s == T - 1))
                # normalize: num = ps_b[:, 0:128], den = ps_b[:, 130]
                den = wp.tile([128, 1], F32, tag="den")
                nc.vector.tensor_scalar(out=den[:], in0=ps_b[:, HID + 2:HID + 3],
                                        scalar1=0.0, scalar2=None,
                                        op0=mybir.AluOpType.is_equal)
                nc.vector.tensor_tensor(out=den[:], in0=den[:],
                                        in1=ps_b[:, HID + 2:HID + 3],
                                        op=mybir.AluOpType.add)
                rec = wp.tile([128, 1], F32, tag="rec")
                nc.vector.reciprocal(out=rec[:], in_=den[:])
                ob = obp.tile([128, HID], F32, tag="ob")
                nc.vector.tensor_scalar(out=ob[:], in0=ps_b[:, 0:HID], scalar1=rec[:],
                                        scalar2=None, op0=mybir.AluOpType.mult)
                nc.sync.dma_start(agg[t * 128:(t + 1) * 128, :], ob[:])
                sq = obp.tile([128, HID], F32, tag="sq")
                nc.scalar.activation(out=sq[:], in_=ob[:],
                                     func=mybir.ActivationFunctionType.Square)
                nc.tensor.matmul(out=ps_sum[:], lhsT=ones_col[:], rhs=ob[:],
                                 start=(t == 0), stop=(t == NBLK - 1))
                nc.tensor.matmul(out=ps_sq[:], lhsT=ones_col[:], rhs=sq[:],
                                 start=(t == 0), stop=(t == NBLK - 1))
                q0 += T
            st_sb = cp.tile([1, 256], F32)
            nc.vector.tensor_copy(out=st_sb[:, 0:128], in_=ps_sum[:])
            nc.vector.tensor_copy(out=st_sb[:, 128:256], in_=ps_sq[:])
            nc.sync.dma_start(stats[:], st_sb[:])
    nc.compile()
    return nc


# ---------------------------------------------------------------- node tail
def build_node2(classifier):
    """BN apply + relu (+ next-layer table build, or classifier)."""
    nc = bacc.Bacc("TRN2", target_bir_lowering=False, debug=False, num_devices=NCORE)
    agg = nc.dram_tensor("agg", [PC, HID], F32, kind="ExternalInput")
    parts = nc.dram_tensor("parts", [8, 256], F32, kind="ExternalInput")
    gb = nc.dram_tensor("gb", [1, 256], F32, kind="ExternalInput")  # [gamma|beta]
    if classifier:
        Wn = nc.dram_tensor("Wn", [128, NCLS], F32, kind="ExternalInput")
        bc = nc.dram_tensor("bc", [1, NCLS], F32, kind="ExternalInput")
        out = nc.dram_tensor("out", [PC, NCLS], F32, kind="ExternalOutput")
    else:
        Wn = nc.dram_tensor("Wn", [128, HID], F32, kind="ExternalInput")
        avec = nc.dram_tensor("avec", [128, 2], F32, kind="ExternalInput")
        out = nc.dram_tensor("out", [PC, TCOLS], F32, kind="ExternalOutput")

    with tile.TileContext(nc) as tc:
        with (
            tc.tile_pool(name="c", bufs=1) as cp,
            tc.tile_pool(name="x", bufs=3) as xp,
            tc.tile_pool(name="o", bufs=3) as op,
            tc.tile_pool(name="ps", bufs=2, space="PSUM") as pp,
            tc.tile_pool(name="ps1", bufs=2, space="PSUM") as pp1,
        ):
            ident = cp.tile([128, 128], F32)
            make_identity(nc, ident[:])
            parts_sb = cp.tile([8, 256], F32)
            nc.sync.dma_start(parts_sb[:], parts[:])
            ones8 = cp.tile([8, 1], F32)
            nc.vector.memset(ones8[:], 1.0)
            st_ps = pp1.tile([1, 256], F32, tag="tmp")
            nc.tensor.matmul(out=st_ps[:], lhsT=ones8[:], rhs=parts_sb[:], start=True, stop=True)
            stat = cp.tile([1, 256], F32)
            nc.vector.tensor_scalar(out=stat[:], in0=st_ps[:], scalar1=1.0 / N,
                                    scalar2=None, op0=mybir.AluOpType.mult)
            mean = stat[:, 0:128]
            msq = stat[:, 128:256]
            var = cp.tile([1, 128], F32)
            nc.vector.tensor_tensor(out=var[:], in0=mean, in1=mean, op=mybir.AluOpType.mult)
            nc.vector.tensor_tensor(out=var[:], in0=msq, in1=var[:], op=mybir.AluOpType.subtract)
            nc.vector.tensor_scalar(out=var[:], in0=var[:], scalar1=EPS,
                                    scalar2=None, op0=mybir.AluOpType.add)
            std = cp.tile([1, 128], F32)
            nc.scalar.activation(out=std[:], in_=var[:],
                                 func=mybir.ActivationFunctionType.Sqrt)
            istd = cp.tile([1, 128], F32)
            nc.vector.reciprocal(out=istd[:], in_=std[:])
            gb_sb = cp.tile([1, 256], F32)
            nc.sync.dma_start(gb_sb[:], gb[:])
            gam = cp.tile([1, 128], F32)
            nc.vector.tensor_tensor(out=gam[:], in0=gb_sb[:, 0:128], in1=istd[:],
                                    op=mybir.AluOpType.mult)
            bet = cp.tile([1, 128], F32)
            nc.vector.tensor_tensor(out=bet[:], in0=mean, in1=gam[:], op=mybir.AluOpType.mult)
            nc.vector.tensor_tensor(out=bet[:], in0=gb_sb[:, 128:256], in1=bet[:],
                                    op=mybir.AluOpType.subtract)
            # broadcast gamma', beta' to [128, 128] via K=1 matmul
            one1 = cp.tile([1, 128], F32)
            nc.vector.memset(one1[:], 1.0)
            gbc_ps = pp1.tile([128, 128], F32, tag="tmp")
            nc.tensor.matmul(out=gbc_ps[:], lhsT=one1[:], rhs=gam[:], start=True, stop=True)
            gbc = cp.tile([128, 128], F32)
            nc.vector.tensor_copy(out=gbc[:], in_=gbc_ps[:])
            bbc_ps = pp1.tile([128, 128], F32, tag="tmp")
            nc.tensor.matmul(out=bbc_ps[:], lhsT=one1[:], rhs=bet[:], start=True, stop=True)
            bbc = cp.tile([128, 128], F32)
            nc.vector.tensor_copy(out=bbc[:], in_=bbc_ps[:])

            if classifier:
                wn_sb = cp.tile([128, NCLS], F32)
                nc.sync.dma_start(wn_sb[:], Wn[:])
                bc_sb = cp.tile([1, NCLS], F32)
                nc.sync.dma_start(bc_sb[:], bc[:])
                bcb_ps = pp1.tile([128, NCLS], F32, tag="tmp")
                nc.tensor.matmul(out=bcb_ps[:], lhsT=one1[:], rhs=bc_sb[:], start=True, stop=True)
                bcb = cp.tile([128, NCLS], F32)
                nc.vector.tensor_copy(out=bcb[:], in_=bcb_ps[:])
                rhs_w = wn_sb
                ncols = NCLS
            else:
                wn_sb = cp.tile([128, HID], F32)
                nc.sync.dma_start(wn_sb[:], Wn[:])
                av_sb = cp.tile([128, 2], F32)
                nc.sync.dma_start(av_sb[:], avec[:])
                wT_ps = pp1.tile([128, 128], F32, tag="tmp")
                nc.tensor.transpose(out=wT_ps[:], in_=wn_sb[:], identity=ident[:])
                wT_sb = cp.tile([128, 128], F32)
                nc.vector.tensor_copy(out=wT_sb[:], in_=wT_ps[:])
                v_ps = pp1.tile([128, 2], F32, tag="tmp")
                nc.tensor.matmul(out=v_ps[:], lhsT=wT_sb[:], rhs=av_sb[:], start=True, stop=True)
                waug = cp.tile([128, HID + 2], F32)
                nc.vector.tensor_copy(out=waug[:, 0:HID], in_=wn_sb[:])
                nc.vector.tensor_copy(out=waug[:, HID:HID + 2], in_=v_ps[:])
                rhs_w = waug
                ncols = HID + 2

            for t in range(NBLK):
                at = xp.tile([128, HID], F32, tag="a")
                nc.sync.dma_start(at[:], agg[t * 128:(t + 1) * 128, :])
                x2 = xp.tile([128, HID], F32, tag="x2")
                nc.vector.tensor_tensor(out=x2[:], in0=at[:], in1=gbc[:], op=mybir.AluOpType.mult)
                nc.vector.tensor_tensor(out=x2[:], in0=x2[:], in1=bbc[:], op=mybir.AluOpType.add)
                nc.scalar.activation(out=x2[:], in_=x2[:],
                                     func=mybir.ActivationFunctionType.Relu)
                xT_ps = pp.tile([128, 128], F32, tag="xt")
                nc.tensor.transpose(out=xT_ps[:], in_=x2[:], identity=ident[:])
                xT_sb = xp.tile([128, 128], F32, tag="xts")
                nc.vector.tensor_copy(out=xT_sb[:], in_=xT_ps[:])
                h_ps = pp.tile([128, ncols], F32, tag="h")
                nc.tensor.matmul(out=h_ps[:], lhsT=xT_sb[:], rhs=rhs_w[:], start=True, stop=True)
                if classifier:
                    ot = op.tile([128, NCLS], F32, tag="o")
                    nc.vector.tensor_tensor(out=ot[:], in0=h_ps[:], in1=bcb[:],
                                            op=mybir.AluOpType.add)
                    nc.scalar.dma_start(out[t * 128:(t + 1) * 128, :], ot[:])
                else:
                    ot = op.tile([128, TCOLS], F32, tag="o")
                    nc.vector.tensor_copy(out=ot[:, 0:HID + 2], in_=h_ps[:])
                    nc.vector.memset(ot[:, HID + 2:HID + 3], 1.0)
                    nc.vector.memset(ot[:, HID + 3:TCOLS], 0.0)
                    nc.scalar.dma_start(out[t * 128:(t + 1) * 128, :], ot[:])
    nc.compile()
    return nc


# ---------------------------------------------------------------- host glue
def _edge_arrays(src, dst):
    """Build per-core src_idx/dst_local arrays + shared t_counts."""
    order = np.argsort(dst, kind="stable")
    srcs = src[order]
    dsts = dst[order]
    blk = (dsts // BLK).astype(np.int64)
    counts = np.bincount(blk, minlength=NPAD // BLK)
    starts = np.concatenate([[0], np.cumsum(counts)])
    # shared subtile counts per slot t: max over cores
    cnt_mat = counts.reshape(NCORE, NBLK)
    t_counts = np.maximum(np.ceil(cnt_mat / BLK).astype(np.int64).max(axis=0), 1)
    nsub = int(t_counts.sum())
    offs = np.concatenate([[0], np.cumsum(t_counts)])
    src_arrs, dst_arrs = [], []
    for c in range(NCORE):
        si = np.zeros((128, nsub), np.int32)
        dl = np.full((128, nsub), 200.0, np.float32)
        for t in range(NBLK):
            b = c * NBLK + t
            s0, e0 = starts[b], starts[b + 1]
            cnt = e0 - s0
            if cnt == 0:
                continue
            k = np.arange(cnt)
            p = k % 128
            q = offs[t] + k // 128
            rolled = (srcs[s0:e0] - PC * c) % NPAD
            si[p, q] = rolled.astype(np.int32)
            dl[p, q] = (dsts[s0:e0] - b * BLK).astype(np.float32)
        src_arrs.append(si)
        dst_arrs.append(dl)
    return t_counts, src_arrs, dst_arrs


_CACHE = {}


def kernel(x, edge_index, W1, as1, ad1, b1, g1, beta1,
           W2, as2, ad2, b2, g2, beta2, Wc, bc):
    x = np.asarray(x, np.float32)
    ei = np.asarray(edge_index)
    src = np.concatenate([ei[0], np.arange(N, dtype=ei.dtype)]).astype(np.int64)
    dst = np.concatenate([ei[1], np.arange(N, dtype=ei.dtype)]).astype(np.int64)

    t_counts, src_arrs, dst_arrs = _edge_arrays(src, dst)

    key = tuple(t_counts.tolist())
    if key not in _CACHE:
        _CACHE[key] = (build_l1(), build_edge(t_counts),
                       build_node2(False), build_node2(True))
    nc1, nce, nc3, nc5 = _CACHE[key]

    # ---- L1
    xT = np.zeros((128, NPAD), np.float32)
    xT[:, :N] = np.asarray(x, np.float32).T
    av = np.stack([np.asarray(as1, np.float32), np.asarray(ad1, np.float32)], axis=1)
    in1 = [{"xT": np.roll(xT, -PC * c, axis=1).copy(),
            "W1": np.asarray(W1, np.float32), "avec": av} for c in range(NCORE)]
    r1 = _run(nc1, in1, "L1")
    h1 = np.concatenate([r1[c]["out"] for c in range(NCORE)], axis=0)  # [NPAD, 132]

    # ---- E1
    ine = [{"table": np.roll(h1, -PC * c, axis=0).copy(),
            "src_idx": src_arrs[c], "dst_loc": dst_arrs[c]} for c in range(NCORE)]
    re1 = _run(nce, ine, "E1")
    agg1 = [re1[c]["agg"] for c in range(NCORE)]
    parts1 = np.stack([re1[c]["stats"][0] for c in range(NCORE)], axis=0)  # [8, 256]

    # ---- L3
    gb1 = np.concatenate([np.asarray(g1, np.float32),
                          np.asarray(beta1, np.float32)])[None, :]
    av2 = np.stack([np.asarray(as2, np.float32), np.asarray(ad2, np.float32)], axis=1)
    in3 = [{"agg": agg1[c], "parts": parts1, "gb": gb1,
            "Wn": np.asarray(W2, np.float32), "avec": av2} for c in range(NCORE)]
    r3 = _run(nc3, in3, "L3")
    h2 = np.concatenate([r3[c]["out"] for c in range(NCORE)], axis=0)

    # ---- E2
    ine2 = [{"table": np.roll(h2, -PC * c, axis=0).copy(),
             "src_idx": src_arrs[c], "dst_loc": dst_arrs[c]} for c in range(NCORE)]
    re2 = _run(nce, ine2, "E2")
    agg2 = [re2[c]["agg"] for c in range(NCORE)]
    parts2 = np.stack([re2[c]["stats"][0] for c in range(NCORE)], axis=0)

    # ---- L5
    gb2 = np.concatenate([np.asarray(g2, np.float32),
                          np.asarray(beta2, np.float32)])[None, :]
    in5 = [{"agg": agg2[c], "parts": parts2, "gb": gb2,
            "Wn": np.asarray(Wc, np.float32),
            "bc": np.asarray(bc, np.float32)[None, :]} for c in range(NCORE)]
    r5 = _run(nc5, in5, "L5")
    logits = np.concatenate([r5[c]["out"] for c in range(NCORE)], axis=0)
    return logits[:N]



# revision 5
# speedup vs baseline: 1.0536x; 1.0536x over previous
"""GAT 2-layer + BN + classifier on 8 TRN2 NeuronCores (Bass/Tile).

v2: dst-block sharding, host-gathered per-edge attention scalars,
block-batched indirect gathers, bf16 matmuls. 5 launches:
  L1   node: h1 = x @ W1 (bf16) -> table shard [h|1] + av=[as|ad]
  E(1) edge: batched gather tab[src] per dst block, one fused DVE op +
       one bf16 matmul per 128-edge subtile, segment softmax via the
       table's ones column, partial BN stats
  L3   node: BN1 apply (transpose + ACT scale/bias/relu) + @W2 -> table2
  E(2) edge: same NEFF as E(1) on table2
  L5   node: BN2 apply + @Wc + bc -> logits shard
Host work is index-only glue: edge sort/shard, per-edge alpha gather
(static indices), table assembly/rotation, weight preprocessing.

All inter-launch tensors use p-major block layout:
  row of node (block t, slot p) lives at partition p, column t*W..(t+1)*W
so every DMA is 128 large contiguous descriptors.
Table rows for the gather are ordered (p * GBLK + b) so row of global
node g in core c's rolled table is (g%128)*GBLK + ((g//128 - 98c) % 784).
"""
import sys
sys.path.insert(0, '/opt/trn_rl_repo')
sys.path.insert(0, '/root/.axon_site')
import numpy as np
import ml_dtypes

import concourse.bass as bass
import concourse.bacc as bacc
import concourse.tile as tile
from concourse import mybir
from concourse.masks import make_identity

F32 = mybir.dt.float32
BF16 = mybir.dt.bfloat16
I32 = mybir.dt.int32
BF = ml_dtypes.bfloat16

N = 100000
NCORE = 8
BLK = 128
NPAD = 100352            # 784 blocks of 128
PC = NPAD // NCORE       # 12544 nodes/core
NBLK = PC // BLK         # 98 dst blocks per core
GBLK = NPAD // BLK       # 784 global blocks
TC = 129                 # table row: [h(128) | 1]
HID = 128
NCLS = 40
NEG = 0.2
EPS = 1e-5
GRP = 7                  # blocks per grouped output DMA (98 = 14*7)
PAD_ALPHA = -1000.0      # exp(lrelu(PAD_ALPHA)) == 0 exactly

_EXEC_NS = []
PROFILE = False
RUN_HOOK = None


def _run(nc, in_maps, label):
    if RUN_HOOK is not None:
        return RUN_HOOK(nc, in_maps, label)
    from concourse import bass2jax
    return bass2jax.run_bass_via_pjrt(nc, in_maps, n_cores=NCORE)


# ---------------------------------------------------------------- L1 node
def build_l1():
    nc = bacc.Bacc("TRN2", target_bir_lowering=False, debug=False, num_devices=NCORE)
    xT = nc.dram_tensor("xT", [128, PC], BF16, kind="ExternalInput")
    waug = nc.dram_tensor("waug", [128, HID + 2], BF16, kind="ExternalInput")
    tab = nc.dram_tensor("tab", [128, NBLK * TC], BF16, kind="ExternalOutput")
    av = nc.dram_tensor("av", [128, NBLK * 2], F32, kind="ExternalOutput")

    with tile.TileContext(nc) as tc:
        with (
            tc.tile_pool(name="c", bufs=1) as cp,
            tc.tile_pool(name="o", bufs=3) as op,
            tc.tile_pool(name="ps", bufs=4, space="PSUM") as pp,
        ):
            w_sb = cp.tile([128, HID + 2], BF16)
            nc.sync.dma_start(w_sb[:], waug[:])
            x_sb = cp.tile([128, PC], BF16)
            nc.sync.dma_start(x_sb[:], xT[:])
            avst = cp.tile([128, NBLK * 2], F32)

            for g0 in range(0, NBLK, GRP):
                ot = op.tile([128, GRP * TC], BF16, tag="o")
                nc.vector.memset(ot[:], 1.0)
                for k in range(GRP):
                    t = g0 + k
                    h_ps = pp.tile([128, HID + 2], F32, tag="h")
                    nc.tensor.matmul(out=h_ps[:], lhsT=x_sb[:, t * 128:(t + 1) * 128],
                                     rhs=w_sb[:], start=True, stop=True)
                    nc.scalar.activation(out=ot[:, k * TC:k * TC + HID],
                                         in_=h_ps[:, 0:HID],
                                         func=mybir.ActivationFunctionType.Copy)
                    nc.vector.tensor_copy(out=avst[:, t * 2:t * 2 + 2],
                                          in_=h_ps[:, HID:HID + 2])
                nc.scalar.dma_start(tab[:, g0 * TC:(g0 + GRP) * TC], ot[:])
            nc.sync.dma_start(av[:], avst[:])
    nc.compile()
    return nc


# ---------------------------------------------------------------- edge kernel
def build_edge(t_counts):
    """t_counts: NBLK subtile counts (shared across cores)."""
    nsub = int(sum(t_counts))
    TMAX = int(max(t_counts))
    nc = bacc.Bacc("TRN2", target_bir_lowering=False, debug=False, num_devices=NCORE)
    tab = nc.dram_tensor("tab", [NPAD, TC], BF16, kind="ExternalInput")
    src_idx = nc.dram_tensor("src_idx", [128, nsub], I32, kind="ExternalInput")
    dst_loc = nc.dram_tensor("dst_loc", [128, nsub], F32, kind="ExternalInput")
    alpha = nc.dram_tensor("alpha", [128, nsub], F32, kind="ExternalInput")
    alphasl = nc.dram_tensor("alphasl", [128, NBLK], F32, kind="ExternalInput")
    agg = nc.dram_tensor("agg", [128, NBLK * HID], BF16, kind="ExternalOutput")
    stats = nc.dram_tensor("stats", [1, 256], F32, kind="ExternalOutput")

    with tile.TileContext(nc) as tc:
        with (
            tc.tile_pool(name="c", bufs=1) as cp,
            tc.tile_pool(name="g", bufs=4) as gp,
            tc.tile_pool(name="sw", bufs=6) as swp,
            tc.tile_pool(name="sq", bufs=3) as sqp,
            tc.tile_pool(name="rc", bufs=6) as rcp,
            tc.tile_pool(name="og", bufs=2) as ogp,
            tc.tile_pool(name="pb", bufs=2, space="PSUM") as pbp,
            tc.tile_pool(name="pst", bufs=1, space="PSUM") as pstp,
        ):
            iota_i = cp.tile([128, 128], I32)
            nc.gpsimd.iota(iota_i[:], pattern=[[1, 128]], base=0, channel_multiplier=0)
            iota_b = cp.tile([128, 128], BF16)
            nc.vector.tensor_copy(out=iota_b[:], in_=iota_i[:])
            ones = cp.tile([128, 1], BF16)
            nc.vector.memset(ones[:], 1.0)
            idx_sb = cp.tile([128, nsub], I32)
            nc.sync.dma_start(idx_sb[:], src_idx[:])
            dl_sb = cp.tile([128, nsub], F32)
            nc.sync.dma_start(dl_sb[:], dst_loc[:])
            al_sb = cp.tile([128, nsub], F32)
            nc.sync.dma_start(al_sb[:], alpha[:])
            asl_sb = cp.tile([128, NBLK], F32)
            nc.sync.dma_start(asl_sb[:], alphasl[:])
            # own-shard table rows (self-loop h): row (p*GBLK + t) for t<NBLK
            ta = tab[:]
            own_ap = bass.AP(ta.tensor, 0, [[GBLK * TC, 128], [TC, NBLK], [1, TC]])
            own_sb = cp.tile([128, NBLK * TC], BF16)
            nc.sync.dma_start(own_sb[:], own_ap)
            # wsl = exp(lrelu(alphasl))
            lsl_sb = cp.tile([128, NBLK], F32)
            nc.vector.tensor_scalar(out=lsl_sb[:], in0=asl_sb[:], scalar1=NEG,
                                    scalar2=None, op0=mybir.AluOpType.mult)
            nc.vector.tensor_tensor(out=lsl_sb[:], in0=lsl_sb[:], in1=asl_sb[:],
                                    op=mybir.AluOpType.max)
            wsl_sb = cp.tile([128, NBLK], F32)
            nc.scalar.activation(out=wsl_sb[:], in_=lsl_sb[:],
                                 func=mybir.ActivationFunctionType.Exp)
            # w = exp(lrelu(alpha)) in bulk: lrelu(x) = max(x, 0.2x)
            lr_sb = cp.tile([128, nsub], F32)
            nc.vector.tensor_scalar(out=lr_sb[:], in0=al_sb[:], scalar1=NEG,
                                    scalar2=None, op0=mybir.AluOpType.mult)
            nc.vector.tensor_tensor(out=lr_sb[:], in0=lr_sb[:], in1=al_sb[:],
                                    op=mybir.AluOpType.max)
            w_sb = cp.tile([128, nsub], F32)
            nc.scalar.activation(out=w_sb[:], in_=lr_sb[:],
                                 func=mybir.ActivationFunctionType.Exp)

            ps_sum = pstp.tile([1, 128], F32, tag="sum")
            ps_sq = pstp.tile([1, 128], F32, tag="sq")

            # zero-init gather buffers (stale-NaN guard)
            for i in range(4):
                gi = gp.tile([128, TMAX * TC], BF16, tag="g", name=f"ginit{i}")
                nc.vector.memset(gi[:], 0.0)

            q0 = 0
            og = None
            for t in range(NBLK):
                T = int(t_counts[t])
                g = gp.tile([128, TMAX * TC], BF16, tag="g")
                for s in range(T):
                    nc.gpsimd.indirect_dma_start(
                        out=g[:, s * TC:(s + 1) * TC], out_offset=None, in_=tab[:],
                        in_offset=bass.IndirectOffsetOnAxis(
                            ap=idx_sb[:, q0 + s:q0 + s + 1], axis=0),
                        bounds_check=NPAD - 1, oob_is_err=False)
                ps = pbp.tile([128, TC], F32, tag="b")
                for s in range(T):
                    q = q0 + s
                    sw = swp.tile([128, 128], BF16, tag="sw")
                    nc.vector.tensor_scalar(
                        out=sw[:], in0=iota_b[:], scalar1=dl_sb[:, q:q + 1],
                        scalar2=w_sb[:, q:q + 1],
                        op0=mybir.AluOpType.is_equal, op1=mybir.AluOpType.mult)
                    nc.tensor.matmul(out=ps[:], lhsT=sw[:], rhs=g[:, s * TC:(s + 1) * TC],
                                     start=(s == 0), stop=(s == T - 1))
                k = t % GRP
                if k == 0:
                    og = ogp.tile([128, GRP * HID], BF16, tag="og")
                num = rcp.tile([128, HID], F32, tag="num")
                nc.vector.scalar_tensor_tensor(
                    out=num[:], in0=own_sb[:, t * TC:t * TC + HID],
                    scalar=wsl_sb[:, t:t + 1], in1=ps[:, 0:HID],
                    op0=mybir.AluOpType.mult, op1=mybir.AluOpType.add)
                den = rcp.tile([128, 1], F32, tag="den")
                nc.vector.tensor_tensor(out=den[:], in0=ps[:, HID:HID + 1],
                                        in1=wsl_sb[:, t:t + 1],
                                        op=mybir.AluOpType.add)
                rc = rcp.tile([128, 1], F32, tag="rc")
                nc.vector.reciprocal(out=rc[:], in_=den[:])
                nc.vector.tensor_scalar(
                    out=og[:, k * HID:(k + 1) * HID], in0=num[:],
                    scalar1=rc[:], scalar2=None, op0=mybir.AluOpType.mult)
                sq = sqp.tile([128, 128], BF16, tag="sq")
                nc.scalar.activation(out=sq[:], in_=og[:, k * HID:(k + 1) * HID],
                                     func=mybir.ActivationFunctionType.Square)
                nc.tensor.matmul(out=ps_sum[:], lhsT=ones[:],
                                 rhs=og[:, k * HID:(k + 1) * HID],
                                 start=(t == 0), stop=(t == NBLK - 1))
                nc.tensor.matmul(out=ps_sq[:], lhsT=ones[:], rhs=sq[:],
                                 start=(t == 0), stop=(t == NBLK - 1))
                if k == GRP - 1:
                    g0 = t - (GRP - 1)
                    nc.scalar.dma_start(agg[:, g0 * HID:(t + 1) * HID], og[:])
                q0 += T

            st = cp.tile([1, 256], F32)
            nc.vector.tensor_copy(out=st[:, 0:128], in_=ps_sum[:])
            nc.vector.tensor_copy(out=st[:, 128:256], in_=ps_sq[:])
            nc.sync.dma_start(stats[:], st[:])
    nc.compile()
    return nc


# ---------------------------------------------------------------- node tail
def build_node2(classifier):
    """BN apply + relu + matmul (next-layer table or classifier)."""
    nc = bacc.Bacc("TRN2", target_bir_lowering=False, debug=False, num_devices=NCORE)
    agg = nc.dram_tensor("agg", [128, NBLK * HID], BF16, kind="ExternalInput")
    parts = nc.dram_tensor("parts", [8, 256], F32, kind="ExternalInput")
    gb = nc.dram_tensor("gb", [1, 256], F32, kind="ExternalInput")  # [gamma|beta]
    if classifier:
        Wn = nc.dram_tensor("Wn", [128, NCLS], BF16, kind="ExternalInput")
        bc = nc.dram_tensor("bc", [1, NCLS], F32, kind="ExternalInput")
        out = nc.dram_tensor("out", [128, NBLK * NCLS], F32, kind="ExternalOutput")
        ncols = NCLS
    else:
        Wn = nc.dram_tensor("Wn", [128, HID + 2], BF16, kind="ExternalInput")
        tab = nc.dram_tensor("tab", [128, NBLK * TC], BF16, kind="ExternalOutput")
        av = nc.dram_tensor("av", [128, NBLK * 2], F32, kind="ExternalOutput")
        ncols = HID + 2

    with tile.TileContext(nc) as tc:
        with (
            tc.tile_pool(name="c", bufs=1) as cp,
            tc.tile_pool(name="x", bufs=4) as xp,
            tc.tile_pool(name="o", bufs=3) as op,
            tc.tile_pool(name="ps", bufs=4, space="PSUM") as pp,
            tc.tile_pool(name="pt", bufs=2, space="PSUM") as ptp,
            tc.tile_pool(name="p1", bufs=2, space="PSUM") as p1p,
        ):
            identb = cp.tile([128, 128], BF16)
            make_identity(nc, identb[:])
            w_sb = cp.tile([128, ncols], BF16)
            nc.sync.dma_start(w_sb[:], Wn[:])
            agg_sb = cp.tile([128, NBLK * HID], BF16)
            nc.sync.dma_start(agg_sb[:], agg[:])

            # BN stats: mean/var from 8-core partials
            parts_sb = cp.tile([8, 256], F32)
            nc.sync.dma_start(parts_sb[:], parts[:])
            ones8 = cp.tile([8, 1], F32)
            nc.vector.memset(ones8[:], 1.0)
            st_ps = p1p.tile([1, 256], F32, tag="t1")
            nc.tensor.matmul(out=st_ps[:], lhsT=ones8[:], rhs=parts_sb[:],
                             start=True, stop=True)
            stat = cp.tile([1, 256], F32)
            nc.vector.tensor_scalar(out=stat[:], in0=st_ps[:], scalar1=1.0 / N,
                                    scalar2=None, op0=mybir.AluOpType.mult)
            mean = stat[:, 0:128]
            msq = stat[:, 128:256]
            var = cp.tile([1, 128], F32)
            nc.vector.tensor_tensor(out=var[:], in0=mean, in1=mean,
                                    op=mybir.AluOpType.mult)
            nc.vector.tensor_tensor(out=var[:], in0=msq, in1=var[:],
                                    op=mybir.AluOpType.subtract)
            nc.vector.tensor_scalar(out=var[:], in0=var[:], scalar1=EPS,
                                    scalar2=None, op0=mybir.AluOpType.add)
            std = cp.tile([1, 128], F32)
            nc.scalar.activation(out=std[:], in_=var[:],
                                 func=mybir.ActivationFunctionType.Sqrt)
            istd = cp.tile([1, 128], F32)
            nc.vector.reciprocal(out=istd[:], in_=std[:])
            gb_sb = cp.tile([1, 256], F32)
            nc.sync.dma_start(gb_sb[:], gb[:])
            gam = cp.tile([1, 128], F32)
            nc.vector.tensor_tensor(out=gam[:], in0=gb_sb[:, 0:128], in1=istd[:],
                                    op=mybir.AluOpType.mult)
            bet = cp.tile([1, 128], F32)
            nc.vector.tensor_tensor(out=bet[:], in0=mean, in1=gam[:],
                                    op=mybir.AluOpType.mult)
            nc.vector.tensor_tensor(out=bet[:], in0=gb_sb[:, 128:256], in1=bet[:],
                                    op=mybir.AluOpType.subtract)
            # gamma'/beta' as per-partition columns via K=1 matmul transpose
            one1 = cp.tile([1, 1], F32)
            nc.vector.memset(one1[:], 1.0)
            gcol_ps = p1p.tile([128, 1], F32, tag="t1")
            nc.tensor.matmul(out=gcol_ps[:], lhsT=gam[:], rhs=one1[:],
                             start=True, stop=True)
            gcol = cp.tile([128, 1], F32)
            nc.vector.tensor_copy(out=gcol[:], in_=gcol_ps[:])
            bcol_ps = p1p.tile([128, 1], F32, tag="t1")
            nc.tensor.matmul(out=bcol_ps[:], lhsT=bet[:], rhs=one1[:],
                             start=True, stop=True)
            bcol = cp.tile([128, 1], F32)
            nc.vector.tensor_copy(out=bcol[:], in_=bcol_ps[:])

            if classifier:
                bc_sb = cp.tile([1, NCLS], F32)
                nc.sync.dma_start(bc_sb[:], bc[:])
                one1b = cp.tile([1, 128], F32)
                nc.vector.memset(one1b[:], 1.0)
                bcb_ps = p1p.tile([128, NCLS], F32, tag="t1")
                nc.tensor.matmul(out=bcb_ps[:], lhsT=one1b[:], rhs=bc_sb[:],
                                 start=True, stop=True)
                bcb = cp.tile([128, NCLS], F32)
                nc.vector.tensor_copy(out=bcb[:], in_=bcb_ps[:])
            else:
                avst = cp.tile([128, NBLK * 2], F32)

            OW = NCLS if classifier else TC
            for g0 in range(0, NBLK, GRP):
                if classifier:
                    ot = op.tile([128, GRP * NCLS], F32, tag="o")
                else:
                    ot = op.tile([128, GRP * TC], BF16, tag="o")
                    nc.vector.memset(ot[:], 1.0)
                for k in range(GRP):
                    t = g0 + k
                    tr_ps = ptp.tile([128, 128], BF16, tag="tr")
                    nc.tensor.transpose(out=tr_ps[:],
                                        in_=agg_sb[:, t * HID:(t + 1) * HID],
                                        identity=identb[:])
                    x2 = xp.tile([128, 128], BF16, tag="x2")
                    nc.scalar.activation(out=x2[:], in_=tr_ps[:],
                                         func=mybir.ActivationFunctionType.Relu,
                                         bias=bcol[:], scale=gcol[:])
                    h_ps = pp.tile([128, ncols], F32, tag="h")
                    nc.tensor.matmul(out=h_ps[:], lhsT=x2[:], rhs=w_sb[:],
                                     start=True, stop=True)
                    if classifier:
                        nc.vector.tensor_tensor(out=ot[:, k * NCLS:(k + 1) * NCLS],
                                                in0=h_ps[:], in1=bcb[:],
                                                op=mybir.AluOpType.add)
                    else:
                        nc.scalar.activation(out=ot[:, k * TC:k * TC + HID],
                                             in_=h_ps[:, 0:HID],
                                             func=mybir.ActivationFunctionType.Copy)
                        nc.vector.tensor_copy(out=avst[:, t * 2:t * 2 + 2],
                                              in_=h_ps[:, HID:HID + 2])
                dst = out if classifier else tab
                nc.scalar.dma_start(dst[:, g0 * OW:(g0 + GRP) * OW], ot[:])
            if not classifier:
                nc.sync.dma_start(av[:], avst[:])
    nc.compile()
    return nc


# ---------------------------------------------------------------- host glue
def _edge_arrays(src, dst):
    """Per-core src_idx/dst_local/position arrays + shared t_counts.

    src_idx values address the p-major rolled table of the owning core:
      row(g) = (g % 128) * GBLK + ((g // 128 - NBLK*c) % GBLK)
    """
    order = np.argsort(dst, kind="stable")
    srcs = dst_sorted_src = src[order]
    dsts = dst[order]
    blk = dsts // BLK
    counts = np.bincount(blk, minlength=GBLK)
    starts = np.concatenate([[0], np.cumsum(counts)])
    cnt_mat = counts.reshape(NCORE, NBLK)
    t_counts = np.maximum(
        np.ceil(cnt_mat / BLK).astype(np.int64).max(axis=0), 1)
    nsub = int(t_counts.sum())
    offs = np.concatenate([[0], np.cumsum(t_counts)])
    cores = []
    for c in range(NCORE):
        si = np.full((128, nsub), 1 << 20, np.int32)
        dlv = np.full((128, nsub), 200.0, np.float32)
        pos_p, pos_q, e_s, e_d = [], [], [], []
        for t in range(NBLK):
            b = c * NBLK + t
            s0, e0 = int(starts[b]), int(starts[b + 1])
            cnt = e0 - s0
            if cnt == 0:
                continue
            k = np.arange(cnt)
            p = k % 128
            q = offs[t] + k // 128
            sg = srcs[s0:e0]
            dg = dsts[s0:e0]
            si[p, q] = ((sg % BLK) * GBLK
                        + (sg // BLK - NBLK * c) % GBLK).astype(np.int32)
            dlv[p, q] = (dg - b * BLK).astype(np.float32)
            pos_p.append(p)
            pos_q.append(q)
            e_s.append(sg)
            e_d.append(dg)
        cores.append(dict(
            si=si, dl=dlv,
            pp=np.concatenate(pos_p), pq=np.concatenate(pos_q),
            es=np.concatenate(e_s), ed=np.concatenate(e_d)))
    return t_counts, nsub, cores


def _assemble_tables(tab_outs):
    """Per-core L-launch table shards [128, NBLK*TC] -> rolled p-major
    full tables [NPAD, TC] per core."""
    tab3 = np.empty((128, GBLK, TC), dtype=BF)
    for c in range(NCORE):
        tab3[:, c * NBLK:(c + 1) * NBLK, :] = \
            tab_outs[c].reshape(128, NBLK, TC)
    tabs = []
    for c in range(NCORE):
        r = np.roll(tab3, -NBLK * c, axis=1)
        tabs.append(np.ascontiguousarray(r).reshape(NPAD, TC))
    return tabs


def _assemble_av(av_outs):
    """Per-core av shards [128, NBLK*2] f32 -> global asv/adv [NPAD]."""
    asv = np.empty(NPAD, np.float32)
    adv = np.empty(NPAD, np.float32)
    for c in range(NCORE):
        a3 = av_outs[c].reshape(128, NBLK, 2)
        asv.reshape(GBLK, 128)[c * NBLK:(c + 1) * NBLK] = a3[:, :, 0].T
        adv.reshape(GBLK, 128)[c * NBLK:(c + 1) * NBLK] = a3[:, :, 1].T
    return asv, adv


def _alphasl_arrays(asv, adv):
    """Self-loop alpha per core: [128, NBLK] f32, node (t,p) of core c."""
    outs = []
    a = (asv + adv).reshape(NCORE, NBLK, 128)
    for c in range(NCORE):
        outs.append(np.ascontiguousarray(a[c].T))
    return outs


def _alpha_arrays(asv, adv, cores, nsub):
    outs = []
    for core in cores:
        A = np.full((128, nsub), PAD_ALPHA, np.float32)
        A[core['pp'], core['pq']] = asv[core['es']] + adv[core['ed']]
        outs.append(A)
    return outs


_CACHE = {}


def kernel(x, edge_index, W1, as1, ad1, b1, g1, beta1,
           W2, as2, ad2, b2, g2, beta2, Wc, bc):
    x = np.asarray(x, np.float32)
    ei = np.asarray(edge_index)
    src = ei[0].astype(np.int64)
    dst = ei[1].astype(np.int64)

    t_counts, nsub, cores = _edge_arrays(src, dst)
    key = tuple(t_counts.tolist())
    if key not in _CACHE:
        _CACHE[key] = (build_l1(), build_edge(t_counts),
                       build_node2(False), build_node2(True))
    nc1, nce, nc3, nc5 = _CACHE[key]

    W1 = np.asarray(W1, np.float32)
    W2 = np.asarray(W2, np.float32)
    Wc = np.asarray(Wc, np.float32)
    g1 = np.asarray(g1, np.float32)
    beta1 = np.asarray(beta1, np.float32)
    g2 = np.asarray(g2, np.float32)
    beta2 = np.asarray(beta2, np.float32)

    # ---- L1
    xp = np.zeros((NPAD, 128), np.float32)
    xp[:N] = x
    waug1 = np.concatenate([W1, (W1 @ np.asarray(as1, np.float32))[:, None],
                            (W1 @ np.asarray(ad1, np.float32))[:, None]],
                           axis=1).astype(BF)
    in1 = [{"xT": np.ascontiguousarray(xp[c * PC:(c + 1) * PC].T).astype(BF),
            "waug": waug1} for c in range(NCORE)]
    r1 = _run(nc1, in1, "L1")

    tabs1 = _assemble_tables([r1[c]["tab"] for c in range(NCORE)])
    asv1, adv1 = _assemble_av([r1[c]["av"] for c in range(NCORE)])
    alphas1 = _alpha_arrays(asv1, adv1, cores, nsub)
    asl1 = _alphasl_arrays(asv1, adv1)

    # ---- E1
    ine = [{"tab": tabs1[c], "src_idx": cores[c]['si'],
            "dst_loc": cores[c]['dl'], "alpha": alphas1[c],
            "alphasl": asl1[c]}
           for c in range(NCORE)]
    re1 = _run(nce, ine, "E1")
    agg1 = [re1[c]["agg"] for c in range(NCORE)]
    parts1 = np.stack([re1[c]["stats"][0] for c in range(NCORE)], axis=0)

    # ---- L3
    gb1 = np.concatenate([g1, beta1])[None, :]
    waug2 = np.concatenate([W2, (W2 @ np.asarray(as2, np.float32))[:, None],
                            (W2 @ np.asarray(ad2, np.float32))[:, None]],
                           axis=1).astype(BF)
    in3 = [{"agg": agg1[c], "parts": parts1, "gb": gb1, "Wn": waug2}
           for c in range(NCORE)]
    r3 = _run(nc3, in3, "L3")

    tabs2 = _assemble_tables([r3[c]["tab"] for c in range(NCORE)])
    asv2, adv2 = _assemble_av([r3[c]["av"] for c in range(NCORE)])
    alphas2 = _alpha_arrays(asv2, adv2, cores, nsub)
    asl2 = _alphasl_arrays(asv2, adv2)

    # ---- E2
    ine2 = [{"tab": tabs2[c], "src_idx": cores[c]['si'],
             "dst_loc": cores[c]['dl'], "alpha": alphas2[c],
             "alphasl": asl2[c]}
            for c in range(NCORE)]
    re2 = _run(nce, ine2, "E2")
    agg2 = [re2[c]["agg"] for c in range(NCORE)]
    parts2 = np.stack([re2[c]["stats"][0] for c in range(NCORE)], axis=0)

    # pad-node correction for BN2 stats: pad nodes aggregate to
    # x2_pad = relu(BN1(0)) and h2_pad = x2_pad @ W2, included (NPAD - N)
    # times in the E2 partial sums but absent from the reference mean.
    s1 = parts1.sum(axis=0)
    mu1 = s1[0:128] / N
    var1 = s1[128:256] / N - mu1 * mu1
    x2pad = np.maximum((0.0 - mu1) / np.sqrt(var1 + EPS) * g1 + beta1, 0.0)
    h2pad = x2pad.astype(BF).astype(np.float32) @ \
        waug2[:, 0:HID].astype(np.float32)
    npad_extra = NPAD - N
    parts2 = parts2.copy()
    parts2[0, 0:128] -= npad_extra * h2pad
    parts2[0, 128:256] -= npad_extra * h2pad * h2pad

    # ---- L5
    gb2 = np.concatenate([g2, beta2])[None, :]
    in5 = [{"agg": agg2[c], "parts": parts2, "gb": gb2,
            "Wn": Wc.astype(BF),
            "bc": np.asarray(bc, np.float32)[None, :]} for c in range(NCORE)]
    r5 = _run(nc5, in5, "L5")

    logits = np.empty((NPAD, NCLS), np.float32)
    l4 = logits.reshape(NCORE, NBLK, 128, NCLS)
    for c in range(NCORE):
        l4[c] = r5[c]["out"].reshape(128, NBLK, NCLS).transpose(1, 0, 2)
    return logits[:N]


# revision 6
# speedup vs baseline: 1.1041x; 1.0479x over previous
"""GAT 2-layer + BN + classifier on 8 TRN2 NeuronCores (Bass/Tile).

v2: dst-block sharding, host-gathered per-edge attention scalars,
block-batched indirect gathers, bf16 matmuls. 5 launches:
  L1   node: h1 = x @ W1 (bf16) -> table shard [h|1] + av=[as|ad]
  E(1) edge: batched gather tab[src] per dst block, one fused DVE op +
       one bf16 matmul per 128-edge subtile, segment softmax via the
       table's ones column, partial BN stats
  L3   node: BN1 apply (transpose + ACT scale/bias/relu) + @W2 -> table2
  E(2) edge: same NEFF as E(1) on table2
  L5   node: BN2 apply + @Wc + bc -> logits shard
Host work is index-only glue: edge sort/shard, per-edge alpha gather
(static indices), table assembly/rotation, weight preprocessing.

All inter-launch tensors use p-major block layout:
  row of node (block t, slot p) lives at partition p, column t*W..(t+1)*W
so every DMA is 128 large contiguous descriptors.
Table rows for the gather are ordered (p * GBLK + b) so row of global
node g in core c's rolled table is (g%128)*GBLK + ((g//128 - 98c) % 784).
"""
import sys
sys.path.insert(0, '/opt/trn_rl_repo')
sys.path.insert(0, '/root/.axon_site')
import numpy as np
import ml_dtypes

import concourse.bass as bass
import concourse.bacc as bacc
import concourse.tile as tile
from concourse import mybir
from concourse.masks import make_identity

F32 = mybir.dt.float32
BF16 = mybir.dt.bfloat16
I32 = mybir.dt.int32
BF = ml_dtypes.bfloat16

N = 100000
NCORE = 8
BLK = 128
NPAD = 100352            # 784 blocks of 128
PC = NPAD // NCORE       # 12544 nodes/core
NBLK = PC // BLK         # 98 dst blocks per core
GBLK = NPAD // BLK       # 784 global blocks
TC = 129                 # table row: [h(128) | 1]
HID = 128
NCLS = 40
NEG = 0.2
EPS = 1e-5
GRP = 7                  # blocks per grouped output DMA (98 = 14*7)
PAD_ALPHA = -1000.0      # exp(lrelu(PAD_ALPHA)) == 0 exactly

_EXEC_NS = []
PROFILE = False
RUN_HOOK = None


def _run(nc, in_maps, label):
    if RUN_HOOK is not None:
        return RUN_HOOK(nc, in_maps, label)
    from concourse import bass2jax
    return bass2jax.run_bass_via_pjrt(nc, in_maps, n_cores=NCORE)


# ---------------------------------------------------------------- L1 node
def build_l1():
    nc = bacc.Bacc("TRN2", target_bir_lowering=False, debug=False, num_devices=NCORE)
    xT = nc.dram_tensor("xT", [128, PC], BF16, kind="ExternalInput")
    waug = nc.dram_tensor("waug", [128, HID + 2], BF16, kind="ExternalInput")
    tab = nc.dram_tensor("tab", [128, NBLK * TC], BF16, kind="ExternalOutput")
    av = nc.dram_tensor("av", [128, NBLK * 2], F32, kind="ExternalOutput")

    with tile.TileContext(nc) as tc:
        with (
            tc.tile_pool(name="c", bufs=1) as cp,
            tc.tile_pool(name="o", bufs=3) as op,
            tc.tile_pool(name="ps", bufs=4, space="PSUM") as pp,
        ):
            w_sb = cp.tile([128, HID + 2], BF16)
            nc.sync.dma_start(w_sb[:], waug[:])
            x_sb = cp.tile([128, PC], BF16)
            nc.sync.dma_start(x_sb[:], xT[:])
            avst = cp.tile([128, NBLK * 2], F32)

            for g0 in range(0, NBLK, GRP):
                ot = op.tile([128, GRP * TC], BF16, tag="o")
                nc.vector.memset(ot[:], 1.0)
                for k in range(GRP):
                    t = g0 + k
                    h_ps = pp.tile([128, HID + 2], F32, tag="h")
                    nc.tensor.matmul(out=h_ps[:], lhsT=x_sb[:, t * 128:(t + 1) * 128],
                                     rhs=w_sb[:], start=True, stop=True)
                    if t % 2 == 0:
                        nc.scalar.activation(out=ot[:, k * TC:k * TC + HID],
                                             in_=h_ps[:, 0:HID],
                                             func=mybir.ActivationFunctionType.Copy)
                    else:
                        nc.vector.tensor_copy(out=ot[:, k * TC:k * TC + HID],
                                              in_=h_ps[:, 0:HID])
                    nc.vector.tensor_copy(out=avst[:, t * 2:t * 2 + 2],
                                          in_=h_ps[:, HID:HID + 2])
                nc.scalar.dma_start(tab[:, g0 * TC:(g0 + GRP) * TC], ot[:])
            nc.sync.dma_start(av[:], avst[:])
    nc.compile()
    return nc


# ---------------------------------------------------------------- edge kernel
def build_edge(t_counts):
    """t_counts: NBLK subtile counts (shared across cores)."""
    nsub = int(sum(t_counts))
    TMAX = int(max(t_counts))
    nc = bacc.Bacc("TRN2", target_bir_lowering=False, debug=False, num_devices=NCORE)
    tab = nc.dram_tensor("tab", [NPAD, TC], BF16, kind="ExternalInput")
    src_idx = nc.dram_tensor("src_idx", [128, nsub], I32, kind="ExternalInput")
    dst_loc = nc.dram_tensor("dst_loc", [128, nsub], F32, kind="ExternalInput")
    alpha = nc.dram_tensor("alpha", [128, nsub], F32, kind="ExternalInput")
    alphasl = nc.dram_tensor("alphasl", [128, NBLK], F32, kind="ExternalInput")
    agg = nc.dram_tensor("agg", [128, NBLK * HID], BF16, kind="ExternalOutput")
    stats = nc.dram_tensor("stats", [1, 256], F32, kind="ExternalOutput")

    with tile.TileContext(nc) as tc:
        with (
            tc.tile_pool(name="c", bufs=1) as cp,
            tc.tile_pool(name="g", bufs=8) as gp,
            tc.tile_pool(name="sw", bufs=8) as swp,
            tc.tile_pool(name="sq", bufs=3) as sqp,
            tc.tile_pool(name="rc", bufs=6) as rcp,
            tc.tile_pool(name="og", bufs=2) as ogp,
            tc.tile_pool(name="pb", bufs=2, space="PSUM") as pbp,
            tc.tile_pool(name="pst", bufs=1, space="PSUM") as pstp,
        ):
            iota_i = cp.tile([128, 128], I32)
            nc.gpsimd.iota(iota_i[:], pattern=[[1, 128]], base=0, channel_multiplier=0)
            iota_b = cp.tile([128, 128], BF16)
            nc.vector.tensor_copy(out=iota_b[:], in_=iota_i[:])
            ones = cp.tile([128, 1], BF16)
            nc.vector.memset(ones[:], 1.0)
            idx_sb = cp.tile([128, nsub], I32)
            nc.sync.dma_start(idx_sb[:], src_idx[:])
            dl_sb = cp.tile([128, nsub], F32)
            nc.sync.dma_start(dl_sb[:], dst_loc[:])
            al_sb = cp.tile([128, nsub], F32)
            nc.sync.dma_start(al_sb[:], alpha[:])
            asl_sb = cp.tile([128, NBLK], F32)
            nc.sync.dma_start(asl_sb[:], alphasl[:])
            # own-shard table rows (self-loop h): row (p*GBLK + t) for t<NBLK
            ta = tab[:]
            own_ap = bass.AP(ta.tensor, 0, [[GBLK * TC, 128], [TC, NBLK], [1, TC]])
            own_sb = cp.tile([128, NBLK * TC], BF16)
            nc.sync.dma_start(own_sb[:], own_ap)
            # wsl = exp(lrelu(alphasl))
            lsl_sb = cp.tile([128, NBLK], F32)
            nc.vector.tensor_scalar(out=lsl_sb[:], in0=asl_sb[:], scalar1=NEG,
                                    scalar2=None, op0=mybir.AluOpType.mult)
            nc.vector.tensor_tensor(out=lsl_sb[:], in0=lsl_sb[:], in1=asl_sb[:],
                                    op=mybir.AluOpType.max)
            wsl_sb = cp.tile([128, NBLK], F32)
            nc.scalar.activation(out=wsl_sb[:], in_=lsl_sb[:],
                                 func=mybir.ActivationFunctionType.Exp)
            # w = exp(lrelu(alpha)) in bulk: lrelu(x) = max(x, 0.2x)
            lr_sb = cp.tile([128, nsub], F32)
            nc.vector.tensor_scalar(out=lr_sb[:], in0=al_sb[:], scalar1=NEG,
                                    scalar2=None, op0=mybir.AluOpType.mult)
            nc.vector.tensor_tensor(out=lr_sb[:], in0=lr_sb[:], in1=al_sb[:],
                                    op=mybir.AluOpType.max)
            w_sb = cp.tile([128, nsub], F32)
            nc.scalar.activation(out=w_sb[:], in_=lr_sb[:],
                                 func=mybir.ActivationFunctionType.Exp)

            ps_sum = pstp.tile([1, 128], F32, tag="sum")
            ps_sq = pstp.tile([1, 128], F32, tag="sq")

            # zero-init gather buffers (stale-NaN guard)
            for i in range(8):
                gi = gp.tile([128, TMAX * TC], BF16, tag="g", name=f"ginit{i}")
                nc.vector.memset(gi[:], 0.0)

            q0 = 0
            og = None
            for t in range(NBLK):
                T = int(t_counts[t])
                g = gp.tile([128, TMAX * TC], BF16, tag="g")
                for s in range(T):
                    nc.gpsimd.indirect_dma_start(
                        out=g[:, s * TC:(s + 1) * TC], out_offset=None, in_=tab[:],
                        in_offset=bass.IndirectOffsetOnAxis(
                            ap=idx_sb[:, q0 + s:q0 + s + 1], axis=0))
                ps = pbp.tile([128, TC], F32, tag="b")
                for s in range(T):
                    q = q0 + s
                    sw = swp.tile([128, 128], BF16, tag="sw")
                    nc.vector.tensor_scalar(
                        out=sw[:], in0=iota_b[:], scalar1=dl_sb[:, q:q + 1],
                        scalar2=w_sb[:, q:q + 1],
                        op0=mybir.AluOpType.is_equal, op1=mybir.AluOpType.mult)
                    nc.tensor.matmul(out=ps[:], lhsT=sw[:], rhs=g[:, s * TC:(s + 1) * TC],
                                     start=(s == 0), stop=(s == T - 1))
                k = t % GRP
                if k == 0:
                    og = ogp.tile([128, GRP * HID], BF16, tag="og")
                num = rcp.tile([128, HID], F32, tag="num")
                nc.vector.scalar_tensor_tensor(
                    out=num[:], in0=own_sb[:, t * TC:t * TC + HID],
                    scalar=wsl_sb[:, t:t + 1], in1=ps[:, 0:HID],
                    op0=mybir.AluOpType.mult, op1=mybir.AluOpType.add)
                den = rcp.tile([128, 1], F32, tag="den")
                nc.vector.tensor_tensor(out=den[:], in0=ps[:, HID:HID + 1],
                                        in1=wsl_sb[:, t:t + 1],
                                        op=mybir.AluOpType.add)
                rc = rcp.tile([128, 1], F32, tag="rc")
                nc.vector.reciprocal(out=rc[:], in_=den[:])
                nc.vector.tensor_scalar(
                    out=og[:, k * HID:(k + 1) * HID], in0=num[:],
                    scalar1=rc[:], scalar2=None, op0=mybir.AluOpType.mult)
                sq = sqp.tile([128, 128], BF16, tag="sq")
                nc.scalar.activation(out=sq[:], in_=og[:, k * HID:(k + 1) * HID],
                                     func=mybir.ActivationFunctionType.Square)
                nc.tensor.matmul(out=ps_sum[:], lhsT=ones[:],
                                 rhs=og[:, k * HID:(k + 1) * HID],
                                 start=(t == 0), stop=(t == NBLK - 1))
                nc.tensor.matmul(out=ps_sq[:], lhsT=ones[:], rhs=sq[:],
                                 start=(t == 0), stop=(t == NBLK - 1))
                if k == GRP - 1:
                    g0 = t - (GRP - 1)
                    nc.scalar.dma_start(agg[:, g0 * HID:(t + 1) * HID], og[:])
                q0 += T

            st = cp.tile([1, 256], F32)
            nc.vector.tensor_copy(out=st[:, 0:128], in_=ps_sum[:])
            nc.vector.tensor_copy(out=st[:, 128:256], in_=ps_sq[:])
            nc.sync.dma_start(stats[:], st[:])
    nc.compile()
    return nc


# ---------------------------------------------------------------- node tail
def build_node2(classifier):
    """BN apply + relu + matmul (next-layer table or classifier)."""
    nc = bacc.Bacc("TRN2", target_bir_lowering=False, debug=False, num_devices=NCORE)
    agg = nc.dram_tensor("agg", [128, NBLK * HID], BF16, kind="ExternalInput")
    parts = nc.dram_tensor("parts", [8, 256], F32, kind="ExternalInput")
    gb = nc.dram_tensor("gb", [1, 256], F32, kind="ExternalInput")  # [gamma|beta]
    if classifier:
        Wn = nc.dram_tensor("Wn", [128, NCLS], BF16, kind="ExternalInput")
        bc = nc.dram_tensor("bc", [1, NCLS], F32, kind="ExternalInput")
        out = nc.dram_tensor("out", [128, NBLK * NCLS], F32, kind="ExternalOutput")
        ncols = NCLS
    else:
        Wn = nc.dram_tensor("Wn", [128, HID + 2], BF16, kind="ExternalInput")
        tab = nc.dram_tensor("tab", [128, NBLK * TC], BF16, kind="ExternalOutput")
        av = nc.dram_tensor("av", [128, NBLK * 2], F32, kind="ExternalOutput")
        ncols = HID + 2

    with tile.TileContext(nc) as tc:
        with (
            tc.tile_pool(name="c", bufs=1) as cp,
            tc.tile_pool(name="x", bufs=4) as xp,
            tc.tile_pool(name="o", bufs=3) as op,
            tc.tile_pool(name="ps", bufs=4, space="PSUM") as pp,
            tc.tile_pool(name="pt", bufs=2, space="PSUM") as ptp,
            tc.tile_pool(name="p1", bufs=2, space="PSUM") as p1p,
        ):
            identb = cp.tile([128, 128], BF16)
            make_identity(nc, identb[:])
            w_sb = cp.tile([128, ncols], BF16)
            nc.sync.dma_start(w_sb[:], Wn[:])
            agg_sb = cp.tile([128, NBLK * HID], BF16)
            nc.sync.dma_start(agg_sb[:], agg[:])

            # BN stats: mean/var from 8-core partials
            parts_sb = cp.tile([8, 256], F32)
            nc.sync.dma_start(parts_sb[:], parts[:])
            ones8 = cp.tile([8, 1], F32)
            nc.vector.memset(ones8[:], 1.0)
            st_ps = p1p.tile([1, 256], F32, tag="t1")
            nc.tensor.matmul(out=st_ps[:], lhsT=ones8[:], rhs=parts_sb[:],
                             start=True, stop=True)
            stat = cp.tile([1, 256], F32)
            nc.vector.tensor_scalar(out=stat[:], in0=st_ps[:], scalar1=1.0 / N,
                                    scalar2=None, op0=mybir.AluOpType.mult)
            mean = stat[:, 0:128]
            msq = stat[:, 128:256]
            var = cp.tile([1, 128], F32)
            nc.vector.tensor_tensor(out=var[:], in0=mean, in1=mean,
                                    op=mybir.AluOpType.mult)
            nc.vector.tensor_tensor(out=var[:], in0=msq, in1=var[:],
                                    op=mybir.AluOpType.subtract)
            nc.vector.tensor_scalar(out=var[:], in0=var[:], scalar1=EPS,
                                    scalar2=None, op0=mybir.AluOpType.add)
            std = cp.tile([1, 128], F32)
            nc.scalar.activation(out=std[:], in_=var[:],
                                 func=mybir.ActivationFunctionType.Sqrt)
            istd = cp.tile([1, 128], F32)
            nc.vector.reciprocal(out=istd[:], in_=std[:])
            gb_sb = cp.tile([1, 256], F32)
            nc.sync.dma_start(gb_sb[:], gb[:])
            gam = cp.tile([1, 128], F32)
            nc.vector.tensor_tensor(out=gam[:], in0=gb_sb[:, 0:128], in1=istd[:],
                                    op=mybir.AluOpType.mult)
            bet = cp.tile([1, 128], F32)
            nc.vector.tensor_tensor(out=bet[:], in0=mean, in1=gam[:],
                                    op=mybir.AluOpType.mult)
            nc.vector.tensor_tensor(out=bet[:], in0=gb_sb[:, 128:256], in1=bet[:],
                                    op=mybir.AluOpType.subtract)
            # gamma'/beta' as per-partition columns via K=1 matmul transpose
            one1 = cp.tile([1, 1], F32)
            nc.vector.memset(one1[:], 1.0)
            gcol_ps = p1p.tile([128, 1], F32, tag="t1")
            nc.tensor.matmul(out=gcol_ps[:], lhsT=gam[:], rhs=one1[:],
                             start=True, stop=True)
            gcol = cp.tile([128, 1], F32)
            nc.vector.tensor_copy(out=gcol[:], in_=gcol_ps[:])
            bcol_ps = p1p.tile([128, 1], F32, tag="t1")
            nc.tensor.matmul(out=bcol_ps[:], lhsT=bet[:], rhs=one1[:],
                             start=True, stop=True)
            bcol = cp.tile([128, 1], F32)
            nc.vector.tensor_copy(out=bcol[:], in_=bcol_ps[:])

            if classifier:
                bc_sb = cp.tile([1, NCLS], F32)
                nc.sync.dma_start(bc_sb[:], bc[:])
                one1b = cp.tile([1, 128], F32)
                nc.vector.memset(one1b[:], 1.0)
                bcb_ps = p1p.tile([128, NCLS], F32, tag="t1")
                nc.tensor.matmul(out=bcb_ps[:], lhsT=one1b[:], rhs=bc_sb[:],
                                 start=True, stop=True)
                bcb = cp.tile([128, NCLS], F32)
                nc.vector.tensor_copy(out=bcb[:], in_=bcb_ps[:])
            else:
                avst = cp.tile([128, NBLK * 2], F32)

            OW = NCLS if classifier else TC
            for g0 in range(0, NBLK, GRP):
                if classifier:
                    ot = op.tile([128, GRP * NCLS], F32, tag="o")
                else:
                    ot = op.tile([128, GRP * TC], BF16, tag="o")
                    nc.vector.memset(ot[:], 1.0)
                for k in range(GRP):
                    t = g0 + k
                    tr_ps = ptp.tile([128, 128], BF16, tag="tr")
                    nc.tensor.transpose(out=tr_ps[:],
                                        in_=agg_sb[:, t * HID:(t + 1) * HID],
                                        identity=identb[:])
                    x2 = xp.tile([128, 128], BF16, tag="x2")
                    nc.scalar.activation(out=x2[:], in_=tr_ps[:],
                                         func=mybir.ActivationFunctionType.Relu,
                                         bias=bcol[:], scale=gcol[:])
                    h_ps = pp.tile([128, ncols], F32, tag="h")
                    nc.tensor.matmul(out=h_ps[:], lhsT=x2[:], rhs=w_sb[:],
                                     start=True, stop=True)
                    if classifier:
                        nc.vector.tensor_tensor(out=ot[:, k * NCLS:(k + 1) * NCLS],
                                                in0=h_ps[:], in1=bcb[:],
                                                op=mybir.AluOpType.add)
                    else:
                        if t % 2 == 0:
                            nc.scalar.activation(
                                out=ot[:, k * TC:k * TC + HID],
                                in_=h_ps[:, 0:HID],
                                func=mybir.ActivationFunctionType.Copy)
                        else:
                            nc.vector.tensor_copy(out=ot[:, k * TC:k * TC + HID],
                                                  in_=h_ps[:, 0:HID])
                        nc.vector.tensor_copy(out=avst[:, t * 2:t * 2 + 2],
                                              in_=h_ps[:, HID:HID + 2])
                dst = out if classifier else tab
                nc.scalar.dma_start(dst[:, g0 * OW:(g0 + GRP) * OW], ot[:])
            if not classifier:
                nc.sync.dma_start(av[:], avst[:])
    nc.compile()
    return nc


# ---------------------------------------------------------------- host glue
def _edge_arrays(src, dst):
    """Per-core src_idx/dst_local/position arrays + shared t_counts.

    src_idx values address the p-major rolled table of the owning core:
      row(g) = (g % 128) * GBLK + ((g // 128 - NBLK*c) % GBLK)
    """
    order = np.argsort(dst, kind="stable")
    srcs = dst_sorted_src = src[order]
    dsts = dst[order]
    blk = dsts // BLK
    counts = np.bincount(blk, minlength=GBLK)
    starts = np.concatenate([[0], np.cumsum(counts)])
    cnt_mat = counts.reshape(NCORE, NBLK)
    t_counts = np.maximum(
        np.ceil(cnt_mat / BLK).astype(np.int64).max(axis=0), 1)
    nsub = int(t_counts.sum())
    offs = np.concatenate([[0], np.cumsum(t_counts)])
    cores = []
    for c in range(NCORE):
        si = np.zeros((128, nsub), np.int32)
        dlv = np.full((128, nsub), 200.0, np.float32)
        pos_p, pos_q, e_s, e_d = [], [], [], []
        for t in range(NBLK):
            b = c * NBLK + t
            s0, e0 = int(starts[b]), int(starts[b + 1])
            cnt = e0 - s0
            if cnt == 0:
                continue
            k = np.arange(cnt)
            p = k % 128
            q = offs[t] + k // 128
            sg = srcs[s0:e0]
            dg = dsts[s0:e0]
            si[p, q] = ((sg % BLK) * GBLK
                        + (sg // BLK - NBLK * c) % GBLK).astype(np.int32)
            dlv[p, q] = (dg - b * BLK).astype(np.float32)
            pos_p.append(p)
            pos_q.append(q)
            e_s.append(sg)
            e_d.append(dg)
        cores.append(dict(
            si=si, dl=dlv,
            pp=np.concatenate(pos_p), pq=np.concatenate(pos_q),
            es=np.concatenate(e_s), ed=np.concatenate(e_d)))
    return t_counts, nsub, cores


def _assemble_tables(tab_outs):
    """Per-core L-launch table shards [128, NBLK*TC] -> rolled p-major
    full tables [NPAD, TC] per core."""
    tab3 = np.empty((128, GBLK, TC), dtype=BF)
    for c in range(NCORE):
        tab3[:, c * NBLK:(c + 1) * NBLK, :] = \
            tab_outs[c].reshape(128, NBLK, TC)
    tabs = []
    for c in range(NCORE):
        r = np.roll(tab3, -NBLK * c, axis=1)
        tabs.append(np.ascontiguousarray(r).reshape(NPAD, TC))
    return tabs


def _assemble_av(av_outs):
    """Per-core av shards [128, NBLK*2] f32 -> global asv/adv [NPAD]."""
    asv = np.empty(NPAD, np.float32)
    adv = np.empty(NPAD, np.float32)
    for c in range(NCORE):
        a3 = av_outs[c].reshape(128, NBLK, 2)
        asv.reshape(GBLK, 128)[c * NBLK:(c + 1) * NBLK] = a3[:, :, 0].T
        adv.reshape(GBLK, 128)[c * NBLK:(c + 1) * NBLK] = a3[:, :, 1].T
    return asv, adv


def _alphasl_arrays(asv, adv):
    """Self-loop alpha per core: [128, NBLK] f32, node (t,p) of core c."""
    outs = []
    a = (asv + adv).reshape(NCORE, NBLK, 128)
    for c in range(NCORE):
        outs.append(np.ascontiguousarray(a[c].T))
    return outs


def _alpha_arrays(asv, adv, cores, nsub):
    outs = []
    for core in cores:
        A = np.full((128, nsub), PAD_ALPHA, np.float32)
        A[core['pp'], core['pq']] = asv[core['es']] + adv[core['ed']]
        outs.append(A)
    return outs


_CACHE = {}


def kernel(x, edge_index, W1, as1, ad1, b1, g1, beta1,
           W2, as2, ad2, b2, g2, beta2, Wc, bc):
    x = np.asarray(x, np.float32)
    ei = np.asarray(edge_index)
    src = ei[0].astype(np.int64)
    dst = ei[1].astype(np.int64)

    t_counts, nsub, cores = _edge_arrays(src, dst)
    key = tuple(t_counts.tolist())
    if key not in _CACHE:
        _CACHE[key] = (build_l1(), build_edge(t_counts),
                       build_node2(False), build_node2(True))
    nc1, nce, nc3, nc5 = _CACHE[key]

    W1 = np.asarray(W1, np.float32)
    W2 = np.asarray(W2, np.float32)
    Wc = np.asarray(Wc, np.float32)
    g1 = np.asarray(g1, np.float32)
    beta1 = np.asarray(beta1, np.float32)
    g2 = np.asarray(g2, np.float32)
    beta2 = np.asarray(beta2, np.float32)

    # ---- L1
    xp = np.zeros((NPAD, 128), np.float32)
    xp[:N] = x
    waug1 = np.concatenate([W1, (W1 @ np.asarray(as1, np.float32))[:, None],
                            (W1 @ np.asarray(ad1, np.float32))[:, None]],
                           axis=1).astype(BF)
    in1 = [{"xT": np.ascontiguousarray(xp[c * PC:(c + 1) * PC].T).astype(BF),
            "waug": waug1} for c in range(NCORE)]
    r1 = _run(nc1, in1, "L1")

    tabs1 = _assemble_tables([r1[c]["tab"] for c in range(NCORE)])
    asv1, adv1 = _assemble_av([r1[c]["av"] for c in range(NCORE)])
    alphas1 = _alpha_arrays(asv1, adv1, cores, nsub)
    asl1 = _alphasl_arrays(asv1, adv1)

    # ---- E1
    ine = [{"tab": tabs1[c], "src_idx": cores[c]['si'],
            "dst_loc": cores[c]['dl'], "alpha": alphas1[c],
            "alphasl": asl1[c]}
           for c in range(NCORE)]
    re1 = _run(nce, ine, "E1")
    agg1 = [re1[c]["agg"] for c in range(NCORE)]
    parts1 = np.stack([re1[c]["stats"][0] for c in range(NCORE)], axis=0)

    # ---- L3
    gb1 = np.concatenate([g1, beta1])[None, :]
    waug2 = np.concatenate([W2, (W2 @ np.asarray(as2, np.float32))[:, None],
                            (W2 @ np.asarray(ad2, np.float32))[:, None]],
                           axis=1).astype(BF)
    in3 = [{"agg": agg1[c], "parts": parts1, "gb": gb1, "Wn": waug2}
           for c in range(NCORE)]
    r3 = _run(nc3, in3, "L3")

    tabs2 = _assemble_tables([r3[c]["tab"] for c in range(NCORE)])
    asv2, adv2 = _assemble_av([r3[c]["av"] for c in range(NCORE)])
    alphas2 = _alpha_arrays(asv2, adv2, cores, nsub)
    asl2 = _alphasl_arrays(asv2, adv2)

    # ---- E2
    ine2 = [{"tab": tabs2[c], "src_idx": cores[c]['si'],
             "dst_loc": cores[c]['dl'], "alpha": alphas2[c],
             "alphasl": asl2[c]}
            for c in range(NCORE)]
    re2 = _run(nce, ine2, "E2")
    agg2 = [re2[c]["agg"] for c in range(NCORE)]
    parts2 = np.stack([re2[c]["stats"][0] for c in range(NCORE)], axis=0)

    # pad-node correction for BN2 stats: pad nodes aggregate to
    # x2_pad = relu(BN1(0)) and h2_pad = x2_pad @ W2, included (NPAD - N)
    # times in the E2 partial sums but absent from the reference mean.
    s1 = parts1.sum(axis=0)
    mu1 = s1[0:128] / N
    var1 = s1[128:256] / N - mu1 * mu1
    x2pad = np.maximum((0.0 - mu1) / np.sqrt(var1 + EPS) * g1 + beta1, 0.0)
    h2pad = x2pad.astype(BF).astype(np.float32) @ \
        waug2[:, 0:HID].astype(np.float32)
    npad_extra = NPAD - N
    parts2 = parts2.copy()
    parts2[0, 0:128] -= npad_extra * h2pad
    parts2[0, 128:256] -= npad_extra * h2pad * h2pad

    # ---- L5
    gb2 = np.concatenate([g2, beta2])[None, :]
    in5 = [{"agg": agg2[c], "parts": parts2, "gb": gb2,
            "Wn": Wc.astype(BF),
            "bc": np.asarray(bc, np.float32)[None, :]} for c in range(NCORE)]
    r5 = _run(nc5, in5, "L5")

    logits = np.empty((NPAD, NCLS), np.float32)
    l4 = logits.reshape(NCORE, NBLK, 128, NCLS)
    for c in range(NCORE):
        l4[c] = r5[c]["out"].reshape(128, NBLK, NCLS).transpose(1, 0, 2)
    return logits[:N]
